# revision 7
# baseline (speedup 1.0000x reference)
"""AdaConv2d on 8 TRN2 NeuronCores — Winograd F(4,3) along W.

Per-sample adaptive 3x3 conv (stride 1, pad 1): sample b uses
kernel_base * kernel_mask[demog_label[b]].

Data-parallel over batch (8 samples/core). The host computes the
Winograd input transform V = B^T d (per 4-wide output tile along W) and
the per-sample transformed weights W[i,kh] = sum_kw G[i,kw] k[kh,kw]
(the label gather is also host-side). The device computes, per
(sample, oc-block), 6 Winograd planes M[i] = sum_kh W[i,kh] @ V[i,kh]
as PSUM-accumulated matmuls (2.0x fewer PE columns than direct conv),
drains each 3-plane PSUM group with a single strided ScalarE ACTIVATE,
and applies the inverse transform A^T M on the DVE as fused plane-pair
bf16 ops (2x tensor_tensor / 4x tensor_scalar). Output goes out
phase-split [q][h][j] (w = 4j+q) in bf16; the host de-interleaves and
casts to f32.

v3 edge optimizations (trace-driven):
- V DRAM layout [BS, IC, HP, A, NT] (rows outermost) so row-range
  slices are one contiguous run per partition; s0 arrives as two
  row-halves on the sync HWDGE ring and s0's W as three per-ocb slices
  on the scalar HWDGE ring. Ring order keeps prefetches (v1/v2 behind
  s0's slices on the same ring) from stealing HBM bandwidth from the
  critical first bytes. Baseline's first weight byte landed ~10.7us
  (SWDGE descgen ~2.3us/transfer + ring bootstrap).
- ~28 dummy matmuls on a gpsimd-memset tile keep the PE busy from
  t~3.8us so the HAM clock-gate reaches K=8/8 (2.4 GHz) at ~7.2us and
  the real matmuls all run warm (baseline ran its first ~22 at 1.2 GHz).
- W DRAM layout [BS, IC, 2, 18, 128] so per-ocb slices are contiguous.
"""

import numpy as np
from ml_dtypes import bfloat16

NCORES = 8
BS = 8              # samples per core
IC, OC, KS = 128, 256, 3
H = W = 56
HP = 58             # h-padded rows
NT = 14             # Winograd tiles along W (4 outputs each)
A = 6               # Winograd input tile size
NKH = 3
RC = 28             # output rows per psum chunk
NC = H // RC        # 2 chunks
FDC = RC * NT       # 392 psum columns per chunk
FDP = H * NT        # 784 columns per full plane
PB = 512            # psum bank stride (f32 elems)
NWARM = 28          # PE warm-up dummy matmuls (HAM -> K=8/8 before real work)

_cached_nc = None

# F(4,3) transform matrices (Lavin), f64 for host-side precision.
BT4 = np.array([
    [4, 0, -5, 0, 1, 0],
    [0, -4, -4, 1, 1, 0],
    [0, 4, -4, -1, 1, 0],
    [0, -2, -1, 2, 1, 0],
    [0, 2, -1, -2, 1, 0],
    [0, 4, 0, -5, 0, 1]], np.float64)
G4 = np.array([
    [1 / 4, 0, 0],
    [-1 / 6, -1 / 6, -1 / 6],
    [-1 / 6, 1 / 6, -1 / 6],
    [1 / 24, 1 / 12, 1 / 6],
    [1 / 24, -1 / 12, 1 / 6],
    [0, 0, 1]], np.float64)


def _build():
    import concourse.mybir as mybir
    import concourse.bacc as bacc
    import concourse.tile as tile

    nc = bacc.Bacc("TRN2", target_bir_lowering=False, debug=False)
    bf = mybir.dt.bfloat16
    f32 = mybir.dt.float32

    v_ext = nc.declare_dram_parameter("v", [BS, IC, HP, A, NT], bf, isOutput=False)
    w_ext = nc.declare_dram_parameter("w", [BS, IC, 2, A * NKH, 128], bf,
                                      isOutput=False)
    out_ext = nc.declare_dram_parameter("out", [BS, 2, 128, 4, FDP], bf, isOutput=True)

    # out-DMA rings: alternate sync/gpsimd (ScalarE keeps only drains —
    # a doorbell waiting on combines would head-of-line-block later drains
    # in ScalarE's strict-FIFO queue). The last pair goes on sync (HWDGE,
    # ~0.6us first-byte vs ~2us SWDGE) to keep the kernel tail short.
    def out_dma(nc, idx, dst, src, last=False):
        eng = nc.sync if last else [nc.sync, nc.gpsimd][idx % 2]
        eng.dma_start(dst, src)

    with tile.TileContext(nc) as tc:
        with (
            tc.tile_pool(name="const", bufs=1) as cpool,
            tc.tile_pool(name="vin", bufs=3) as vpool,
            tc.tile_pool(name="wgt", bufs=3) as wpool,
            tc.tile_pool(name="mst", bufs=3) as mpool,
            tc.tile_pool(name="itm", bufs=2) as ipool,
            tc.tile_pool(name="yst", bufs=4) as ypool,
            tc.tile_pool(name="psum", bufs=2, space="PSUM") as pspool,
            tc.tile_pool(name="wps", bufs=1, space="PSUM") as wpspool,
        ):
            vt = {}
            wt = {}

            # sample 0's V arrives as two contiguous row-halves on sync
            # (HWDGE) and its W as three per-ocb slices on scalar (HWDGE) so
            # the first matmul group only waits for ~0.9MB; later fetches sit
            # behind these on the same rings (in-order per ring). Samples 1-2
            # prefetch up-front (pool depth 3); s+3 is issued at the end of
            # sample s so no DMA waits block the queues mid-kernel.
            def fetch0():
                vt[0] = vpool.tile([IC, HP, A, NT], bf, name="v0", tag="v")
                wt[0] = wpool.tile([IC, 2, A * NKH, 128], bf, name="w0", tag="w")
                nc.sync.dma_start(vt[0][:, 0:30], v_ext[0, :, 0:30])
                nc.scalar.dma_start(wt[0][:, 0, 0:9], w_ext[0, :, 0, 0:9])
                nc.sync.dma_start(vt[0][:, 30:58], v_ext[0, :, 30:58])
                nc.scalar.dma_start(wt[0][:, 0, 9:18], w_ext[0, :, 0, 9:18])
                nc.scalar.dma_start(wt[0][:, 1], w_ext[0, :, 1])

            def fetch(s):
                vt[s] = vpool.tile([IC, HP, A, NT], bf, name=f"v{s}", tag="v")
                wt[s] = wpool.tile([IC, 2, A * NKH, 128], bf, name=f"w{s}", tag="w")
                nc.sync.dma_start(vt[s][:], v_ext[s])
                nc.gpsimd.dma_start(wt[s][:], w_ext[s])

            # warm-up operand: gpsimd memset (gpsimd's queue is free right
            # after the ~3.4us DGE ring bootstrap — no DMA dependency).
            wz = cpool.tile([128, 272], bf)
            nc.gpsimd.memset(wz[:], 0.0)

            fetch0()

            # PE warm-up: dummy matmuls (the psum bank is never read). They
            # keep the PE busy from t~3.8us so the HAM SHORT window fires
            # (~3.4us) and the real matmuls all run at 2.4 GHz instead of
            # the first ~22 at 1.2.
            warm_ps = wpspool.tile([128, 256], f32)
            for _ in range(NWARM):
                nc.tensor.matmul(warm_ps[:], wz[:, 0:128], wz[:, 0:256],
                                 start=True, stop=True)

            # Trigger the one-time ACT_TABLE_LOAD (~2.7us) after the scalar
            # queue's w0 DMA triggers but before the first real drain. Uses
            # wz columns the warm-up matmuls don't read (no false dep).
            nc.scalar.copy(wz[:, 264:272], wz[:, 256:264])

            fetch(1)
            fetch(2)

            ndma = 0
            for s in range(BS):
                for ocb in range(2):
                    ms = mpool.tile([128, A, FDP], bf, name=f"m{s}_{ocb}", tag="m")
                    for c in range(NC):
                        for half in range(2):
                            pst = pspool.tile(
                                [128, NKH, PB], f32,
                                name=f"ps{s}_{ocb}_{c}_{half}", tag="ps",
                            )
                            for i3 in range(NKH):
                                i = half * NKH + i3
                                for kh in range(NKH):
                                    nc.tensor.matmul(
                                        pst[:, i3, 0:FDC],
                                        wt[s][:, ocb, i * NKH + kh, :],
                                        vt[s][:, RC * c + kh: RC * c + kh + RC, i, :],
                                        start=(kh == 0),
                                        stop=(kh == NKH - 1),
                                    )
                            # one strided drain for the whole 3-bank group
                            nc.scalar.copy(
                                ms[:, half * NKH:(half + 1) * NKH,
                                   c * FDC:(c + 1) * FDC],
                                pst[:, :, 0:FDC],
                            )

                    # inverse transform on DVE: fused plane-pair bf16 ops.
                    # W5 layout: [b, a, d, c, d8]; R: [d2, c4]; TU: [t0, u]
                    y = ypool.tile([128, 4, FDP], bf, name=f"y{s}_{ocb}", tag="y")
                    w5 = ipool.tile([128, 5, FDP], bf, name=f"w5_{s}_{ocb}", tag="w5")
                    rr = ipool.tile([128, 2, FDP], bf, name=f"r_{s}_{ocb}", tag="r")
                    tu = ipool.tile([128, 2, FDP], bf, name=f"tu_{s}_{ocb}", tag="tu")
                    # first pair runs per-chunk so DVE starts before all four
                    # drains land; last pair per-chunk for a short kernel tail
                    split = (s == 0 and ocb == 0) or s == BS - 1
                    parts = [slice(0, FDP)] if not split else [
                        slice(c * FDC, (c + 1) * FDC) for c in range(NC)]
                    for pi, sl in enumerate(parts):
                        v = nc.vector
                        v.tensor_sub(w5[:, 0:3:2, sl], ms[:, 1:4:2, sl],
                                     ms[:, 2:5:2, sl])
                        v.tensor_add(w5[:, 1:4:2, sl], ms[:, 1:4:2, sl],
                                     ms[:, 2:5:2, sl])
                        v.tensor_scalar_mul(rr[:, 0, sl], w5[:, 2, sl], 2.0)
                        v.tensor_scalar_mul(rr[:, 1, sl], w5[:, 3, sl], 4.0)
                        v.tensor_scalar_mul(w5[:, 4, sl], w5[:, 2, sl], 8.0)
                        v.tensor_add(tu[:, :, sl], w5[:, 1::-1, sl], w5[:, 3:5, sl])
                        v.tensor_add(y[:, 0:4:3, sl], tu[:, :, sl], ms[:, 0:6:5, sl])
                        out_dma(nc, ndma, out_ext[s, ocb, :, 0:4:3, sl],
                                y[:, 0:4:3, sl], last=(s == BS - 1))
                        ndma += 1
                        v.tensor_add(y[:, 1:3, sl], w5[:, 0:2, sl], rr[:, :, sl])
                        out_dma(nc, ndma, out_ext[s, ocb, :, 1:3, sl],
                                y[:, 1:3, sl], last=(s == BS - 1))
                        ndma += 1
                if s + 3 < BS and ocb == 1:
                    fetch(s + 3)
    nc.compile()
    return nc


def _host_prep(x, demog_label, kernel_base, kernel_mask):
    B = x.shape[0]
    # pad h and w by 1 (h to 58, w to 58)
    xpad = np.zeros((B, IC, HP, HP), np.float32)
    xpad[:, :, 1:H + 1, 1:W + 1] = x
    # input transform: V[b, ic, h, i, j] = sum_t BT4[i,t] xpad[b, ic, h, 4j+t]
    dwin = np.lib.stride_tricks.sliding_window_view(xpad, A, axis=3)[:, :, :, ::4, :]
    V = np.einsum("it,bchjt->bchij", BT4.astype(np.float32), dwin,
                  optimize=True).astype(bfloat16)
    # weight transform: W[b, ic, i*3+kh, oc] = sum_kw G4[i,kw] kb[oc,ic,kh,kw] m[b,ic,kh,kw]
    mg = kernel_mask[demog_label]                        # [B, IC, 3, 3]
    km = np.einsum("ochw,bchw->bochw", kernel_base, mg, optimize=True)
    Wt = np.einsum("iw,bochw->bciho", G4.astype(np.float32),
                   km, optimize=True)                    # [B, IC, A, KH, OC]
    # device layout [B, IC, ocb, i*3+kh, m] so per-ocb DMA slices are
    # contiguous per partition
    Wt = Wt.reshape(B, IC, A * NKH, 2, 128).transpose(0, 1, 3, 2, 4)
    return V, np.ascontiguousarray(Wt).astype(bfloat16)


def _host_post(buf):
    # buf: [B, 2, 128, 4, 784] bf16 -> [B, 256, 56, 56] f32
    B = buf.shape[0]
    r = buf.reshape(B, 2, 128, 4, H, NT).astype(np.float32)
    r = r.transpose(0, 1, 2, 4, 5, 3)                    # [B, 2, 128, 56, 14, 4]
    return np.ascontiguousarray(r.reshape(B, OC, H, W))


def run(inputs, trace=False, **kw):
    from concourse.bass_utils import run_bass_kernel_spmd

    global _cached_nc
    if _cached_nc is None:
        _cached_nc = _build()
    nc = _cached_nc

    x = np.asarray(inputs["x"], dtype=np.float32)
    demog_label = np.asarray(inputs["demog_label"])
    kernel_base = np.asarray(inputs["kernel_base"], dtype=np.float32)
    kernel_mask = np.asarray(inputs["kernel_mask"], dtype=np.float32)

    V, Wt = _host_prep(x, demog_label, kernel_base, kernel_mask)

    in_maps = []
    for c in range(NCORES):
        sl = slice(c * BS, (c + 1) * BS)
        in_maps.append({
            "v": np.ascontiguousarray(V[sl]),
            "w": np.ascontiguousarray(Wt[sl]),
        })

    last_exc = None
    for _attempt in range(3):
        try:
            res = run_bass_kernel_spmd(nc, in_maps, core_ids=list(range(NCORES)),
                                       trace=trace, **kw)
            outs = [_host_post(np.asarray(r["out"])) for r in res.results]
            full = np.concatenate(outs, axis=0)
            return full, res
        except Exception as e:  # transient NRT/device faults: retry
            last_exc = e
            import time
            time.sleep(10)
    raise last_exc


def kernel(**inputs):
    out, _ = run(inputs, trace=False)
    return out
